# revision 15
# baseline (speedup 1.0000x reference)
"""DCNv2 (deformable conv v2) Trainium2 Bass kernel.

Problem: x[8,64,64,128], offset[8,64,64,18], modulation[8,64,64,9],
conv_kernel[3,3,128,256], conv_bias[256] -> out[8,64,64,256].

Sharding: data-parallel over batch B=8, one batch per NeuronCore.

Per-core algorithm:
  1. Build zero-padded bf16 image img[68*68, 128] in DRAM (interior at
     rows/cols [1..64], matching the reference's 66x66 pad; extra rows/cols
     65..67 stay zero so x0+1 column overreach is always in-bounds).
  2. Compute bilinear sample indices (int16) and per-sample corner weights
     on-device from offset/modulation.
  3. For each 128-pixel tile t and tap k: dma_gather 2-pixel-wide rows
     (y0,x0..x0+1) and (y1,x0..x0+1); 4 diag-weighted transpose matmuls
     accumulate modulated bilinear features feats[c=128, pix] in PSUM;
     copy to a big SBUF FEATS laid out by i = s*64 + h' (s = tap row chunk)
     which linearizes the reference's concat/reshape scrambling.
  4. Conv: per output tile T, 9 matmuls (r x j) with lhsT = FEATS slices
     at i = 6T + 3*delta + r, rhs = conv kernel [c,256] per (r,j).
"""

import os
import sys

import numpy as np

sys.path.insert(0, "/opt/trn_rl_repo")

import concourse.bass as bass  # noqa: E402
import concourse.mybir as mybir  # noqa: E402
from concourse.tile import TileContext  # noqa: E402

F32 = mybir.dt.float32
BF16 = mybir.dt.bfloat16
I16 = mybir.dt.int16

H = W = 64
C = 128
F = 256
NK = 9
NPIX = H * W          # 4096 pixels per batch
IW = 68               # padded image width/height (66 needed + 2 slack)
NPIXP = IW * IW       # 4624 padded pixels
NT = NPIX // 128      # 32 pixel tiles
MAGIC = 1.5 * 2.0**23  # fp32 round-to-int trick; sum stays in [2^23,2^24)
CLIP_MAX = 65.0       # reference clips to ih-1 = 65

# ky varies slowest in the reference tap flattening: tap k = 3*kyi + kxi
KY = np.array([k // 3 - 1 for k in range(9)], np.float32)
KX = np.array([k % 3 - 1 for k in range(9)], np.float32)


def _host_consts():
    """Input-independent constant tensors (computed on host, loaded once)."""
    p = np.arange(128)
    t = np.arange(NT)
    k = np.arange(NK)
    # pix-major [128, (t,k)]: pixel P = t*128 + p; h = P//64 = 2t + p//64
    h = 2 * t[None, :, None] + (p[:, None, None] // 64)   # [128, 32, 1]
    w = (p[:, None, None] % 64)                           # [128, 1->32, 1]
    byp = (h + 1 + KY[None, None, :]).astype(np.float32)  # [128,32,9]
    bxp = (np.broadcast_to(w, (128, NT, 1)) + 1 + KX[None, None, :]).astype(
        np.float32
    )
    byp = byp.reshape(128, NT * NK)
    bxp = bxp.reshape(128, NT * NK)

    # 16-part layout [16, (t, k, g)]: pixel P = t*128 + g*16 + q
    q = np.arange(16)[:, None, None, None]
    tt = t[None, :, None, None]
    kk = np.arange(NK)[None, None, :, None]
    g = np.arange(8)[None, None, None, :]
    P = tt * 128 + g * 16 + q
    h16 = P // 64
    w16 = P % 64
    by16 = (h16 + 1 + KY[kk]).astype(np.float32).reshape(16, NT * NK * 8)
    bx16 = (w16 + 1 + KX[kk]).astype(np.float32).reshape(16, NT * NK * 8)

    # 4 horizontally stacked 128x128 identity blocks, bf16
    import ml_dtypes

    eye = np.eye(128, dtype=np.float32)
    id4 = np.concatenate([eye] * 4, axis=1).astype(ml_dtypes.bfloat16)
    return {"byp": byp, "bxp": bxp, "by16": by16, "bx16": bx16, "id4": id4}


def _dummy_out(tc, out, feats, opool):
    nc = tc.nc
    for T in range(NT):
        ot = opool.tile([128, F], F32, name="ot")
        nc.vector.memset(ot[:], 0.0)
        nc.sync.dma_start(out=out[T * 128 : (T + 1) * 128, :], in_=ot[:])


def build_nc():
    from concourse.bacc import Bacc

    nc = Bacc()

    x = nc.dram_tensor("x", [NPIX, C], F32, kind="ExternalInput")
    off = nc.dram_tensor("off", [NPIX, 2 * NK], F32, kind="ExternalInput")
    mod = nc.dram_tensor("mod", [NPIX, NK], F32, kind="ExternalInput")
    ck = nc.dram_tensor("ck", [NK, C, F], F32, kind="ExternalInput")
    byp_d = nc.dram_tensor("byp", [128, NT * NK], F32, kind="ExternalInput")
    bxp_d = nc.dram_tensor("bxp", [128, NT * NK], F32, kind="ExternalInput")
    by16_d = nc.dram_tensor("by16", [16, NT * NK * 8], F32, kind="ExternalInput")
    bx16_d = nc.dram_tensor("bx16", [16, NT * NK * 8], F32, kind="ExternalInput")
    id4_d = nc.dram_tensor("id4", [128, 512], BF16, kind="ExternalInput")
    out = nc.dram_tensor("out", [NPIX, F], F32, kind="ExternalOutput")

    img = nc.dram_tensor("img", [NPIXP, C], BF16, kind="Internal")

    with TileContext(nc) as tc:
        _body(tc, x, off, mod, ck, byp_d, bxp_d, by16_d, bx16_d, id4_d, out, img)
    nc.finalize()
    return nc


def _body(tc, x, off, mod, ck, byp_d, bxp_d, by16_d, bx16_d, id4_d, out, img):
    import contextlib

    KSTAGE = int(os.environ.get("KSTAGE", "4"))
    nc = tc.nc
    ctx = contextlib.ExitStack()
    # persistent (live through the main loops)
    cpool = ctx.enter_context(tc.tile_pool(name="consts", bufs=1))
    # setup-phase scratch
    spool = ctx.enter_context(tc.tile_pool(name="setup", bufs=1))
    wpool = ctx.enter_context(tc.tile_pool(name="work", bufs=14))
    s6pool = ctx.enter_context(tc.tile_pool(name="st16", bufs=6))
    # main-loop rotating tiles
    gpool = ctx.enter_context(tc.tile_pool(name="gather", bufs=3))
    dgpool = ctx.enter_context(tc.tile_pool(name="diag", bufs=3))
    opool = ctx.enter_context(tc.tile_pool(name="outs", bufs=3))
    ppool = ctx.enter_context(tc.tile_pool(name="psum", bufs=4, space="PSUM"))
    p2pool = ctx.enter_context(tc.tile_pool(name="psum2", bufs=2, space="PSUM"))

    alu = mybir.AluOpType
    act_copy = mybir.ActivationFunctionType.Copy

    # ---------------- persistent tiles ----------------
    byp = cpool.tile([128, NT * NK], F32, name="bypt")
    bxp = cpool.tile([128, NT * NK], F32, name="bxpt")
    id4 = cpool.tile([128, 512], BF16, name="id4t")
    kmat = cpool.tile([128, NK * F], BF16, name="kmatt")
    w4b = cpool.tile([128, NT * NK * 4], BF16, name="w4b")
    idx = cpool.tile([128, NT * 18 * 8], I16, name="idx")
    feats = cpool.tile([128, 3 * 192 * 64], BF16, name="featsbuf")

    # ---------------- Stage A: constants + padded image ----------------
    nc.sync.dma_start(out=byp[:], in_=byp_d[:])
    nc.sync.dma_start(out=bxp[:], in_=bxp_d[:])
    nc.sync.dma_start(out=id4[:], in_=id4_d[:])
    for k in range(NK):
        # cast f32 -> bf16 during DMA (SWDGE)
        nc.gpsimd.dma_start(out=kmat[:, k * F : (k + 1) * F], in_=ck[k])

    # zero the padded image (borrow the feats buffer as a zero source),
    # then fill the interior (cast f32->bf16 in flight)
    nc.vector.memset(feats[:, 0:NPIXP], 0.0)
    nc.sync.dma_start(out=img[:], in_=feats[:, 0:NPIXP])
    imgv = img[:].rearrange("(a b) c -> a b c", b=IW)  # [68, 68, 128]
    nc.gpsimd.dma_start(out=imgv[1:65, 1:65, :], in_=x[:])

    if KSTAGE < 2:
        _dummy_out(tc, out, feats, opool)
        ctx.close()
        return

    # ---------------- Stage B1: pix-major corner weights ----------------
    # free layout (t, k) = t*9 + k, tiles [128, 288] f32
    NTK = NT * NK
    offv = off[:].rearrange("(t p) c -> p t c", p=128)  # strided DRAM view
    offp = spool.tile([128, NT, 2 * NK], F32, name="offp")
    nc.sync.dma_start(out=offp[:], in_=offv)
    modv = mod[:].rearrange("(t p) c -> p t c", p=128)
    modp = spool.tile([128, NT, NK], F32, name="modp")
    nc.sync.dma_start(out=modp[:], in_=modv)
    modf = modp[:].rearrange("p t k -> p (t k)")

    def wt(name):
        return wpool.tile([128, NTK], F32, name=name, tag="wta")

    py = wt("py")
    px = wt("px")
    nc.vector.tensor_add(
        py[:], offp[:, :, 0:NK], byp[:].rearrange("p (t k) -> p t k", k=NK)
    )
    nc.vector.tensor_add(
        px[:], offp[:, :, NK : 2 * NK], bxp[:].rearrange("p (t k) -> p t k", k=NK)
    )

    y0f = wt("y0f")
    x0f = wt("x0f")
    nc.vector.tensor_scalar(y0f[:], py[:], -0.5, MAGIC, alu.add, alu.add)
    nc.vector.tensor_scalar(y0f[:], y0f[:], MAGIC, None, alu.subtract)
    nc.vector.tensor_scalar(x0f[:], px[:], -0.5, MAGIC, alu.add, alu.add)
    nc.vector.tensor_scalar(x0f[:], x0f[:], MAGIC, None, alu.subtract)

    y0 = wt("y0")
    x0 = wt("x0")
    x1c = wt("x1c")
    nc.vector.tensor_scalar(y0[:], y0f[:], 0.0, CLIP_MAX, alu.max, alu.min)
    nc.vector.tensor_scalar(x0[:], x0f[:], 0.0, CLIP_MAX, alu.max, alu.min)
    nc.vector.tensor_scalar(x1c[:], x0f[:], 1.0, 0.0, alu.add, alu.max)
    nc.vector.tensor_scalar(x1c[:], x1c[:], CLIP_MAX, None, alu.min)

    pyc = wt("pyc")
    pxc = wt("pxc")
    nc.vector.tensor_scalar(pyc[:], py[:], 0.0, CLIP_MAX, alu.max, alu.min)
    nc.vector.tensor_scalar(pxc[:], px[:], 0.0, CLIP_MAX, alu.max, alu.min)
    ly = wt("ly")
    lx = wt("lx")
    sx = wt("sx")
    nc.vector.tensor_sub(ly[:], pyc[:], y0[:])
    nc.vector.tensor_sub(lx[:], pxc[:], x0[:])
    nc.vector.tensor_sub(sx[:], x1c[:], x0[:])

    # reference corner pairing:
    # (y0,x0):oly*olx  (y1,x0):oly*lx  (y0,x1):ly*olx  (y1,x1):ly*lx
    oly = wt("oly")
    olx = wt("olx")
    nc.vector.tensor_scalar(oly[:], ly[:], -1.0, 1.0, alu.mult, alu.add)
    nc.vector.tensor_scalar(olx[:], lx[:], -1.0, 1.0, alu.mult, alu.add)
    am = wt("am")  # olx*m
    bm = wt("bm")  # lx*m
    nc.vector.tensor_mul(am[:], olx[:], modf)
    nc.vector.tensor_mul(bm[:], lx[:], modf)
    a0 = wt("a0")  # w(y0,x0) = oly*olx*m
    b0 = wt("b0")  # w(y0,x1) = ly*olx*m
    a1 = wt("a1")  # w(y1,x0) = oly*lx*m
    b1 = wt("b1")  # w(y1,x1) = ly*lx*m
    nc.vector.tensor_mul(a0[:], oly[:], am[:])
    nc.vector.tensor_mul(b0[:], ly[:], am[:])
    nc.vector.tensor_mul(a1[:], oly[:], bm[:])
    nc.vector.tensor_mul(b1[:], ly[:], bm[:])

    osx = wt("osx")
    nc.vector.tensor_scalar(osx[:], sx[:], -1.0, 1.0, alu.mult, alu.add)

    # w4 [128, (t,k,cr)] f32, slot order (r0s0, r0s1, r1s0, r1s1); slot1
    # weights are gated by sx (0 when the reference x1 collapsed onto x0)
    w4 = spool.tile([128, NTK, 4], F32, name="w4")
    tmp = wt("tmpw")
    nc.vector.tensor_mul(tmp[:], osx[:], b0[:])
    nc.vector.tensor_add(w4[:, :, 0], tmp[:], a0[:])
    nc.vector.tensor_mul(w4[:, :, 1], sx[:], b0[:])
    nc.vector.tensor_mul(tmp[:], osx[:], b1[:])
    nc.vector.tensor_add(w4[:, :, 2], tmp[:], a1[:])
    nc.vector.tensor_mul(w4[:, :, 3], sx[:], b1[:])
    nc.vector.tensor_copy(w4b[:], w4[:].rearrange("p a b -> p (a b)"))

    # ---------------- Stage B2: gather indices (16-partition layout) -----
    # idx value = y*68 + x0; gather list order i = (2k+row)*128 + p, so the
    # Q7 reads idx i at [i%16, i//16]: partition q=p%16, col=(2k+row)*8+g.
    NCH = 4
    TPC = NT // NCH
    frees = TPC * NK * 8
    for ci in range(NCH):
        def st(name):
            return s6pool.tile([128, frees], F32, name=name, tag="st16")[0:16]

        o16 = s6pool.tile([128, 2 * frees], F32, name="o16", tag="o16")[0:16]
        src = bass.AP(
            off[:].tensor,
            ci * TPC * 128 * 18,
            [[18, 16], [128 * 18, TPC], [16 * 18, 8], [1, 18]],
        )
        nc.sync.dma_start(
            out=o16[:, :].rearrange("q (t g c) -> q t g c", g=8, c=18), in_=src
        )
        o16v = o16[:, :].rearrange("q (t g c) -> q t c g", g=8, c=18)

        by6 = s6pool.tile([128, frees], F32, name="by6", tag="b16")[0:16]
        bx6 = s6pool.tile([128, frees], F32, name="bx6", tag="b16")[0:16]
        cslc = slice(ci * frees, (ci + 1) * frees)
        nc.sync.dma_start(out=by6[:], in_=by16_d[:, cslc])
        nc.sync.dma_start(out=bx6[:], in_=bx16_d[:, cslc])

        py6 = st("py6")
        px6 = st("px6")
        nc.vector.tensor_add(
            py6[:].rearrange("q (t k g) -> q t k g", k=NK, g=8),
            o16v[:, :, 0:NK, :],
            by6[:].rearrange("q (t k g) -> q t k g", k=NK, g=8),
        )
        nc.vector.tensor_add(
            px6[:].rearrange("q (t k g) -> q t k g", k=NK, g=8),
            o16v[:, :, NK : 2 * NK, :],
            bx6[:].rearrange("q (t k g) -> q t k g", k=NK, g=8),
        )
        y0f6 = st("y0f6")
        x0f6 = st("x0f6")
        nc.vector.tensor_scalar(y0f6[:], py6[:], -0.5, MAGIC, alu.add, alu.add)
        nc.vector.tensor_scalar(y0f6[:], y0f6[:], MAGIC, None, alu.subtract)
        nc.vector.tensor_scalar(x0f6[:], px6[:], -0.5, MAGIC, alu.add, alu.add)
        nc.vector.tensor_scalar(x0f6[:], x0f6[:], MAGIC, None, alu.subtract)
        y06 = st("y06")
        x06 = st("x06")
        y16 = st("y16")
        nc.vector.tensor_scalar(y06[:], y0f6[:], 0.0, CLIP_MAX, alu.max, alu.min)
        nc.vector.tensor_scalar(x06[:], x0f6[:], 0.0, CLIP_MAX, alu.max, alu.min)
        nc.vector.tensor_scalar(y16[:], y0f6[:], 1.0, 0.0, alu.add, alu.max)
        nc.vector.tensor_scalar(y16[:], y16[:], CLIP_MAX, None, alu.min)
        i0 = st("i0")
        i1 = st("i1")
        nc.vector.tensor_scalar(i0[:], y06[:], float(IW), None, alu.mult)
        nc.vector.tensor_add(i0[:], i0[:], x06[:])
        nc.vector.tensor_scalar(i1[:], y16[:], float(IW), None, alu.mult)
        nc.vector.tensor_add(i1[:], i1[:], x06[:])

        # idx col = t*144 + k*16 + row*8 + g
        dst = idx[:].rearrange("p (t k r g) -> p t k r g", k=NK, r=2, g=8)
        tsl = slice(ci * TPC, (ci + 1) * TPC)
        nc.vector.tensor_copy(
            dst[0:16, tsl, :, 0, :],
            i0[:].rearrange("q (t k g) -> q t k g", k=NK, g=8),
        )
        nc.vector.tensor_copy(
            dst[0:16, tsl, :, 1, :],
            i1[:].rearrange("q (t k g) -> q t k g", k=NK, g=8),
        )

    # replicate idx rows to all 8 16-partition groups (Q7 cores)
    for gix in range(1, 8):
        nc.sync.dma_start(out=idx[16 * gix : 16 * (gix + 1), :], in_=idx[0:16, :])

    if KSTAGE < 3:
        _dummy_out(tc, out, feats, opool)
        ctx.close()
        return

    # ---------------- Stage C: gather + weighted bilinear transpose ------
    # FEATS col = j*12288 + i*64 + ow, i = s*64 + h'  (tap k = 3s + j)
    img_gsrc = bass.AP(img[:].tensor, 0, [[C, NPIXP - 1], [1, 2 * C]])

    CSUB = int(os.environ.get("KCSUB", "4"))
    for t in range(NT):
        g = gpool.tile([128, 18, 2 * C], BF16, name="gt")
        nc.gpsimd.dma_gather(
            g[:],
            img_gsrc,
            idx[:, t * 144 : (t + 1) * 144],
            num_idxs=2304,
            num_idxs_reg=2304,
            elem_size=2 * C,
            elem_step=C,
            single_packet=False,
        )
        if CSUB < 2:
            continue
        for k in range(NK):
            dg = dgpool.tile([128, 512], BF16, name="dg")
            wslc = w4b[:, (t * NK + k) * 4 : (t * NK + k) * 4 + 4]
            wb = wslc.broadcast_to([128, 4, 128])
            nc.vector.tensor_mul(
                dg[:].rearrange("p (a b) -> p a b", b=128),
                id4[:].rearrange("p (a b) -> p a b", b=128),
                wb,
            )

            if CSUB < 3:
                continue
            pf = ppool.tile([128, 128], F32, name="pfeats")
            for cr, (row, sl) in enumerate(((0, 0), (0, 1), (1, 0), (1, 1))):
                nc.tensor.matmul(
                    pf[:],
                    g[:, 2 * k + row, sl * C : (sl + 1) * C],
                    dg[:, cr * 128 : (cr + 1) * 128],
                    start=(cr == 0),
                    stop=(cr == 3),
                )
            s, j = k // 3, k % 3
            i0 = s * 64 + 2 * t

            def q(i):
                return (i // 6) * 6 + (i % 3) * 2 + ((i % 6) // 3)

            if CSUB < 4:
                continue
            q0, q1 = q(i0), q(i0 + 1)
            base = j * 12288 + q0 * 64
            dstap = bass.AP(
                feats.tensor,
                feats.offset + base,
                [list(feats.ap[0]), [(q1 - q0) * 64, 2], [1, 64]],
            )
            nc.scalar.activation(dstap, pf[:], act_copy)

    if KSTAGE < 4:
        _dummy_out(tc, out, feats, opool)
        ctx.close()
        return

    # ---------------- Stage D: conv ----------------
    for T in range(NT):
        po = p2pool.tile([128, F], F32, name="pout")
        n = 0
        for r in range(3):
            for j in range(3):
                base = j * 12288 + (T * 6 + r * 2) * 64
                lhsT = feats[:, base : base + 128]
                nc.tensor.matmul(
                    po[:],
                    lhsT,
                    kmat[:, (r * 3 + j) * F : (r * 3 + j + 1) * F],
                    start=(n == 0),
                    stop=(n == 8),
                )
                n += 1
        ot = opool.tile([128, F], F32, name="ot")
        nc.vector.tensor_copy(ot[:], po[:])
        nc.sync.dma_start(out=out[T * 128 : (T + 1) * 128, :], in_=ot[:])

    ctx.close()


_CACHED_NC = None


def _get_nc():
    global _CACHED_NC
    if _CACHED_NC is None:
        _CACHED_NC = build_nc()
    return _CACHED_NC


def kernel(x, offset, modulation, conv_kernel, conv_bias):
    from concourse.bass_utils import run_bass_kernel_spmd

    B = x.shape[0]
    consts = _host_consts()
    ck9 = np.ascontiguousarray(
        conv_kernel.reshape(NK, C, F), dtype=np.float32
    )
    in_maps = []
    for b in range(B):
        in_maps.append(
            {
                "x": np.ascontiguousarray(x[b].reshape(NPIX, C), np.float32),
                "off": np.ascontiguousarray(
                    offset[b].reshape(NPIX, 2 * NK), np.float32
                ),
                "mod": np.ascontiguousarray(
                    modulation[b].reshape(NPIX, NK), np.float32
                ),
                "ck": ck9,
                "byp": consts["byp"],
                "bxp": consts["bxp"],
                "by16": consts["by16"],
                "bx16": consts["bx16"],
                "id4": np.asarray(consts["id4"]),
            }
        )
    nc = _get_nc()
    res = run_bass_kernel_spmd(
        nc,
        in_maps,
        core_ids=list(range(B)),
        trace=bool(int(os.environ.get("KERNEL_TRACE", "0"))),
    )
    outs = [res.results[b]["out"].reshape(H, W, F) for b in range(B)]
    result = np.stack(outs, axis=0) + conv_bias[None, None, None, :]
    if getattr(res, "exec_time_ns", None):
        kernel.last_exec_time_ns = res.exec_time_ns
    return result.astype(np.float32)


# revision 20
# speedup vs baseline: 307.1412x; 307.1412x over previous
"""DCNv2 (deformable conv v2) Trainium2 Bass kernel.

Problem: x[8,64,64,128], offset[8,64,64,18], modulation[8,64,64,9],
conv_kernel[3,3,128,256], conv_bias[256] -> out[8,64,64,256].

Sharding: data-parallel over batch B=8, one batch per NeuronCore.

Per-core algorithm:
  1. Build zero-padded bf16 image img[68*68, 128] in DRAM (interior at
     rows/cols [1..64], matching the reference's 66x66 pad; extra rows/cols
     65..67 stay zero so x0+1 column overreach is always in-bounds).
  2. Compute bilinear sample indices (int16) and per-sample corner weights
     on-device from offset/modulation.
  3. For each 128-pixel tile t and tap k: dma_gather 2-pixel-wide rows
     (y0,x0..x0+1) and (y1,x0..x0+1); 4 diag-weighted transpose matmuls
     accumulate modulated bilinear features feats[c=128, pix] in PSUM;
     copy to a big SBUF FEATS laid out by i = s*64 + h' (s = tap row chunk)
     which linearizes the reference's concat/reshape scrambling.
  4. Conv: per output tile T, 9 matmuls (r x j) with lhsT = FEATS slices
     at i = 6T + 3*delta + r, rhs = conv kernel [c,256] per (r,j).
"""

import os
import sys

import numpy as np

sys.path.insert(0, "/opt/trn_rl_repo")

import concourse.bass as bass  # noqa: E402
import concourse.mybir as mybir  # noqa: E402
from concourse.tile import TileContext  # noqa: E402

F32 = mybir.dt.float32
BF16 = mybir.dt.bfloat16
I16 = mybir.dt.int16

H = W = 64
C = 128
F = 256
NK = 9
NPIX = H * W          # 4096 pixels per batch
IW = 68               # padded image width/height (66 needed + 2 slack)
NPIXP = IW * IW       # 4624 padded pixels
NT = NPIX // 128      # 32 pixel tiles
MAGIC = 1.5 * 2.0**23  # fp32 round-to-int trick; sum stays in [2^23,2^24)
CLIP_MAX = 65.0       # reference clips to ih-1 = 65

# ky varies slowest in the reference tap flattening: tap k = 3*kyi + kxi
KY = np.array([k // 3 - 1 for k in range(9)], np.float32)
KX = np.array([k % 3 - 1 for k in range(9)], np.float32)


def _host_consts():
    """Input-independent constant tensors (computed on host, loaded once)."""
    p = np.arange(128)
    t = np.arange(NT)
    k = np.arange(NK)
    # pix-major [128, (t,k)]: pixel P = t*128 + p; h = P//64 = 2t + p//64
    h = 2 * t[None, :, None] + (p[:, None, None] // 64)   # [128, 32, 1]
    w = (p[:, None, None] % 64)                           # [128, 1->32, 1]
    byp = (h + 1 + KY[None, None, :]).astype(np.float32)  # [128,32,9]
    bxp = (np.broadcast_to(w, (128, NT, 1)) + 1 + KX[None, None, :]).astype(
        np.float32
    )
    byp = byp.reshape(128, NT * NK)
    bxp = bxp.reshape(128, NT * NK)

    # 16-part layout [16, (t, k, g)]: pixel P = t*128 + g*16 + q
    q = np.arange(16)[:, None, None, None]
    tt = t[None, :, None, None]
    kk = np.arange(NK)[None, None, :, None]
    g = np.arange(8)[None, None, None, :]
    P = tt * 128 + g * 16 + q
    h16 = P // 64
    w16 = P % 64
    by16 = (h16 + 1 + KY[kk]).astype(np.float32).reshape(16, NT * NK * 8)
    bx16 = (w16 + 1 + KX[kk]).astype(np.float32).reshape(16, NT * NK * 8)

    # 4 horizontally stacked 128x128 identity blocks, bf16
    import ml_dtypes

    eye = np.eye(128, dtype=np.float32)
    id4 = np.concatenate([eye] * 4, axis=1).astype(ml_dtypes.bfloat16)
    return {"byp": byp, "bxp": bxp, "by16": by16, "bx16": bx16, "id4": id4}


def _dummy_out(tc, out, feats, opool):
    nc = tc.nc
    for T in range(NT):
        ot = opool.tile([128, F], F32, name="ot")
        nc.vector.memset(ot[:], 0.0)
        nc.sync.dma_start(out=out[T * 128 : (T + 1) * 128, :], in_=ot[:])


def build_nc():
    from concourse.bacc import Bacc

    nc = Bacc()

    x = nc.dram_tensor("x", [NPIX, C], F32, kind="ExternalInput")
    off = nc.dram_tensor("off", [NPIX, 2 * NK], F32, kind="ExternalInput")
    mod = nc.dram_tensor("mod", [NPIX, NK], F32, kind="ExternalInput")
    ck = nc.dram_tensor("ck", [NK, C, F], F32, kind="ExternalInput")
    byp_d = nc.dram_tensor("byp", [128, NT * NK], F32, kind="ExternalInput")
    bxp_d = nc.dram_tensor("bxp", [128, NT * NK], F32, kind="ExternalInput")
    by16_d = nc.dram_tensor("by16", [16, NT * NK * 8], F32, kind="ExternalInput")
    bx16_d = nc.dram_tensor("bx16", [16, NT * NK * 8], F32, kind="ExternalInput")
    id4_d = nc.dram_tensor("id4", [128, 512], BF16, kind="ExternalInput")
    out = nc.dram_tensor("out", [NPIX, F], F32, kind="ExternalOutput")

    img = nc.dram_tensor("img", [NPIXP, C], BF16, kind="Internal")
    img2 = nc.dram_tensor("img2", [67 * IW, 2 * C], BF16, kind="Internal")

    with TileContext(nc) as tc:
        _body(tc, x, off, mod, ck, byp_d, bxp_d, by16_d, bx16_d, id4_d, out, img, img2)
    nc.finalize()
    return nc


def _body(tc, x, off, mod, ck, byp_d, bxp_d, by16_d, bx16_d, id4_d, out, img, img2):
    import contextlib

    KREPS = int(os.environ.get("KREPS", "1"))
    nc = tc.nc
    ctx = contextlib.ExitStack()
    # persistent (live through the main loops)
    cpool = ctx.enter_context(tc.tile_pool(name="consts", bufs=1))
    # setup-phase scratch
    spool = ctx.enter_context(tc.tile_pool(name="setup", bufs=1))
    wpool = ctx.enter_context(tc.tile_pool(name="work", bufs=14))
    s6pool = ctx.enter_context(tc.tile_pool(name="st16", bufs=6))
    # main-loop rotating tiles
    gpool = ctx.enter_context(tc.tile_pool(name="gather", bufs=3))
    dgpool = ctx.enter_context(tc.tile_pool(name="diag", bufs=3))
    opool = ctx.enter_context(tc.tile_pool(name="outs", bufs=3))
    ppool = ctx.enter_context(tc.tile_pool(name="psum", bufs=4, space="PSUM"))
    p2pool = ctx.enter_context(tc.tile_pool(name="psum2", bufs=2, space="PSUM"))

    alu = mybir.AluOpType
    act_copy = mybir.ActivationFunctionType.Copy

    # ---------------- persistent tiles ----------------
    byp = cpool.tile([128, NT * NK], F32, name="bypt")
    bxp = cpool.tile([128, NT * NK], F32, name="bxpt")
    id4 = cpool.tile([128, 512], BF16, name="id4t")
    kmat = cpool.tile([128, NK * F], BF16, name="kmatt")
    w4b = cpool.tile([128, NT * NK * 4], BF16, name="w4b")
    idx = cpool.tile([128, NT * 9 * 8], I16, name="idx")
    feats = cpool.tile([128, 3 * 192 * 64], BF16, name="featsbuf")

    env_names = None  # placeholder
    # ---------------- Stage A: constants + padded image ----------------
    nc.sync.dma_start(out=byp[:], in_=byp_d[:])
    nc.sync.dma_start(out=bxp[:], in_=bxp_d[:])
    nc.sync.dma_start(out=id4[:], in_=id4_d[:])
    for k in range(NK):
        # cast f32 -> bf16 during DMA (SWDGE)
        nc.gpsimd.dma_start(out=kmat[:, k * F : (k + 1) * F], in_=ck[k])

    # zero the padded image (borrow the feats buffer as a zero source),
    # then fill the interior (cast f32->bf16 in flight)
    rep = 0  # noqa - rep loop below re-executes the full pipeline for timing
    for rep in range(KREPS):
        _pipeline(tc, x, off, mod, out, img, img2, locals())
    ctx.close()


def _pipeline(tc, x, off, mod, out, img, img2, env):
    nc = tc.nc
    alu = mybir.AluOpType
    act_copy = mybir.ActivationFunctionType.Copy
    cpool = env["cpool"]; spool = env["spool"]; wpool = env["wpool"]
    s6pool = env["s6pool"]; gpool = env["gpool"]; dgpool = env["dgpool"]
    opool = env["opool"]; ppool = env["ppool"]; p2pool = env["p2pool"]
    byp = env["byp"]; bxp = env["bxp"]; id4 = env["id4"]; kmat = env["kmat"]
    w4b = env["w4b"]; idx = env["idx"]; feats = env["feats"]
    by16_d = env["by16_d"]; bx16_d = env["bx16_d"]

    nc.vector.memset(feats[:, 0:NPIXP], 0.0)
    nc.sync.dma_start(out=img[:], in_=feats[:, 0:NPIXP])
    imgv = img[:].rearrange("(a b) c -> a b c", b=IW)  # [68, 68, 128]
    nc.gpsimd.dma_start(out=imgv[1:65, 1:65, :], in_=x[:])
    # img2[j, x] = (img[clip(j-1,0,65), x], img[clip(j,0,65), x]) row pairs:
    # gather entry j serves ye = j-1 (ye = clip(floor(py), -1, 65)); j=0 is
    # the low-clip pair (0,0), j=66 the high-clip pair (65,65).
    i2v = img2[:].rearrange("(j xx) (r c) -> j xx r c", xx=IW, r=2)  # [67,68,2,128]
    nc.sync.dma_start(out=i2v[1:66, :, 0, :], in_=imgv[0:65, :, :])
    nc.sync.dma_start(out=i2v[1:66, :, 1, :], in_=imgv[1:66, :, :])
    nc.sync.dma_start(out=i2v[0, :, 0, :], in_=imgv[0, :, :])
    nc.sync.dma_start(out=i2v[0, :, 1, :], in_=imgv[0, :, :])
    nc.sync.dma_start(out=i2v[66, :, 0, :], in_=imgv[65, :, :])
    nc.sync.dma_start(out=i2v[66, :, 1, :], in_=imgv[65, :, :])

    # ---------------- Stage B1: pix-major corner weights ----------------
    # free layout (t, k) = t*9 + k, tiles [128, 288] f32
    NTK = NT * NK
    offv = off[:].rearrange("(t p) c -> p t c", p=128)  # strided DRAM view
    offp = spool.tile([128, NT, 2 * NK], F32, name="offp")
    nc.sync.dma_start(out=offp[:], in_=offv)
    modv = mod[:].rearrange("(t p) c -> p t c", p=128)
    modp = spool.tile([128, NT, NK], F32, name="modp")
    nc.sync.dma_start(out=modp[:], in_=modv)
    modf = modp[:].rearrange("p t k -> p (t k)")

    def wt(name):
        return wpool.tile([128, NTK], F32, name=name, tag="wta")

    py = wt("py")
    px = wt("px")
    nc.vector.tensor_add(
        py[:], offp[:, :, 0:NK], byp[:].rearrange("p (t k) -> p t k", k=NK)
    )
    nc.vector.tensor_add(
        px[:], offp[:, :, NK : 2 * NK], bxp[:].rearrange("p (t k) -> p t k", k=NK)
    )

    y0f = wt("y0f")
    x0f = wt("x0f")
    nc.vector.tensor_scalar(y0f[:], py[:], -0.5, MAGIC, alu.add, alu.add)
    nc.vector.tensor_scalar(y0f[:], y0f[:], MAGIC, None, alu.subtract)
    nc.vector.tensor_scalar(x0f[:], px[:], -0.5, MAGIC, alu.add, alu.add)
    nc.vector.tensor_scalar(x0f[:], x0f[:], MAGIC, None, alu.subtract)

    y0 = wt("y0")
    x0 = wt("x0")
    x1c = wt("x1c")
    nc.vector.tensor_scalar(y0[:], y0f[:], 0.0, CLIP_MAX, alu.max, alu.min)
    nc.vector.tensor_scalar(x0[:], x0f[:], 0.0, CLIP_MAX, alu.max, alu.min)
    nc.vector.tensor_scalar(x1c[:], x0f[:], 1.0, 0.0, alu.add, alu.max)
    nc.vector.tensor_scalar(x1c[:], x1c[:], CLIP_MAX, None, alu.min)

    pyc = wt("pyc")
    pxc = wt("pxc")
    nc.vector.tensor_scalar(pyc[:], py[:], 0.0, CLIP_MAX, alu.max, alu.min)
    nc.vector.tensor_scalar(pxc[:], px[:], 0.0, CLIP_MAX, alu.max, alu.min)
    ly = wt("ly")
    lx = wt("lx")
    sx = wt("sx")
    nc.vector.tensor_sub(ly[:], pyc[:], y0[:])
    nc.vector.tensor_sub(lx[:], pxc[:], x0[:])
    nc.vector.tensor_sub(sx[:], x1c[:], x0[:])

    # reference corner pairing:
    # (y0,x0):oly*olx  (y1,x0):oly*lx  (y0,x1):ly*olx  (y1,x1):ly*lx
    oly = wt("oly")
    olx = wt("olx")
    nc.vector.tensor_scalar(oly[:], ly[:], -1.0, 1.0, alu.mult, alu.add)
    nc.vector.tensor_scalar(olx[:], lx[:], -1.0, 1.0, alu.mult, alu.add)
    am = wt("am")  # olx*m
    bm = wt("bm")  # lx*m
    nc.vector.tensor_mul(am[:], olx[:], modf)
    nc.vector.tensor_mul(bm[:], lx[:], modf)
    a0 = wt("a0")  # w(y0,x0) = oly*olx*m
    b0 = wt("b0")  # w(y0,x1) = ly*olx*m
    a1 = wt("a1")  # w(y1,x0) = oly*lx*m
    b1 = wt("b1")  # w(y1,x1) = ly*lx*m
    nc.vector.tensor_mul(a0[:], oly[:], am[:])
    nc.vector.tensor_mul(b0[:], ly[:], am[:])
    nc.vector.tensor_mul(a1[:], oly[:], bm[:])
    nc.vector.tensor_mul(b1[:], ly[:], bm[:])

    osx = wt("osx")
    nc.vector.tensor_scalar(osx[:], sx[:], -1.0, 1.0, alu.mult, alu.add)

    # w4 [128, (t,k,cr)] f32, slot order (r0x0, r1x0, r0x1, r1x1) matching
    # the img2 gather element layout; x1 slots gated by sx (0 when the
    # reference x1 collapsed onto x0 by clipping)
    w4 = spool.tile([128, NTK, 4], F32, name="w4")
    tmp = wt("tmpw")
    nc.vector.tensor_mul(tmp[:], osx[:], b0[:])
    nc.vector.tensor_add(w4[:, :, 0], tmp[:], a0[:])
    nc.vector.tensor_mul(tmp[:], osx[:], b1[:])
    nc.vector.tensor_add(w4[:, :, 1], tmp[:], a1[:])
    nc.vector.tensor_mul(w4[:, :, 2], sx[:], b0[:])
    nc.vector.tensor_mul(w4[:, :, 3], sx[:], b1[:])
    nc.vector.tensor_copy(w4b[:], w4[:].rearrange("p a b -> p (a b)"))

    # ---------------- Stage B2: gather indices (16-partition layout) -----
    # one idx per sample: idx = (ye+1)*68 + x0, ye = clip(floor(py), -1, 65).
    # gather list order i = k*128 + p per tile, so the Q7 reads idx i at
    # [i%16, i//16]: partition q = p%16, col = k*8 + g (g = p//16).
    NCH = 4
    TPC = NT // NCH
    frees = TPC * NK * 8
    for ci in range(NCH):
        def st(name):
            return s6pool.tile([128, frees], F32, name=name, tag="st16")[0:16]

        o16 = s6pool.tile([128, 2 * frees], F32, name="o16", tag="o16")[0:16]
        src = bass.AP(
            off[:].tensor,
            ci * TPC * 128 * 18,
            [[18, 16], [128 * 18, TPC], [16 * 18, 8], [1, 18]],
        )
        nc.sync.dma_start(
            out=o16[:, :].rearrange("q (t g c) -> q t g c", g=8, c=18), in_=src
        )
        o16v = o16[:, :].rearrange("q (t g c) -> q t c g", g=8, c=18)

        by6 = s6pool.tile([128, frees], F32, name="by6", tag="b16")[0:16]
        bx6 = s6pool.tile([128, frees], F32, name="bx6", tag="b16")[0:16]
        cslc = slice(ci * frees, (ci + 1) * frees)
        nc.sync.dma_start(out=by6[:], in_=by16_d[:, cslc])
        nc.sync.dma_start(out=bx6[:], in_=bx16_d[:, cslc])

        py6 = st("py6")
        px6 = st("px6")
        nc.vector.tensor_add(
            py6[:].rearrange("q (t k g) -> q t k g", k=NK, g=8),
            o16v[:, :, 0:NK, :],
            by6[:].rearrange("q (t k g) -> q t k g", k=NK, g=8),
        )
        nc.vector.tensor_add(
            px6[:].rearrange("q (t k g) -> q t k g", k=NK, g=8),
            o16v[:, :, NK : 2 * NK, :],
            bx6[:].rearrange("q (t k g) -> q t k g", k=NK, g=8),
        )
        y0f6 = st("y0f6")
        x0f6 = st("x0f6")
        nc.vector.tensor_scalar(y0f6[:], py6[:], -0.5, MAGIC, alu.add, alu.add)
        nc.vector.tensor_scalar(y0f6[:], y0f6[:], MAGIC, None, alu.subtract)
        nc.vector.tensor_scalar(x0f6[:], px6[:], -0.5, MAGIC, alu.add, alu.add)
        nc.vector.tensor_scalar(x0f6[:], x0f6[:], MAGIC, None, alu.subtract)
        ye6 = st("ye6")
        x06 = st("x06")
        nc.vector.tensor_scalar(ye6[:], y0f6[:], -1.0, CLIP_MAX, alu.max, alu.min)
        nc.vector.tensor_scalar(x06[:], x0f6[:], 0.0, CLIP_MAX, alu.max, alu.min)
        i0 = st("i0")
        # idx = (ye+1)*68 + x0
        nc.vector.tensor_scalar(i0[:], ye6[:], float(IW), float(IW), alu.mult, alu.add)
        nc.vector.tensor_add(i0[:], i0[:], x06[:])
        nc.vector.tensor_copy(
            idx[0:16, ci * TPC * 72 : (ci + 1) * TPC * 72], i0[:]
        )

    # replicate idx rows to all 8 16-partition groups (Q7 cores)
    for gix in range(1, 8):
        nc.sync.dma_start(out=idx[16 * gix : 16 * (gix + 1), :], in_=idx[0:16, :])

    # ---------------- Stage C: gather + weighted bilinear transpose ------
    # FEATS col = j*12288 + q(i)*64 + ow, i = s*64 + h'  (tap k = 3s + j)
    img_gsrc = bass.AP(img2[:].tensor, 0, [[2 * C, 67 * IW - 1], [1, 4 * C]])

    for t in range(NT):
        g = gpool.tile([128, NK, 4 * C], BF16, name="gt")
        nc.gpsimd.dma_gather(
            g[:],
            img_gsrc,
            idx[:, t * 72 : (t + 1) * 72],
            num_idxs=NK * 128,
            num_idxs_reg=NK * 128,
            elem_size=4 * C,
            elem_step=2 * C,
            single_packet=False,
        )
        for k in range(NK):
            dg = dgpool.tile([128, 512], BF16, name="dg")
            wslc = w4b[:, (t * NK + k) * 4 : (t * NK + k) * 4 + 4]
            wb = wslc.broadcast_to([128, 4, 128])
            nc.vector.tensor_mul(
                dg[:].rearrange("p (a b) -> p a b", b=128),
                id4[:].rearrange("p (a b) -> p a b", b=128),
                wb,
            )

            pf = ppool.tile([128, 128], F32, name="pfeats")
            for cr in range(4):
                nc.tensor.matmul(
                    pf[:],
                    g[:, k, cr * C : (cr + 1) * C],
                    dg[:, cr * 128 : (cr + 1) * 128],
                    start=(cr == 0),
                    stop=(cr == 3),
                )
            s, j = k // 3, k % 3
            i0 = s * 64 + 2 * t

            def q(i):
                return (i // 6) * 6 + (i % 3) * 2 + ((i % 6) // 3)

            q0, q1 = q(i0), q(i0 + 1)
            base = j * 12288 + q0 * 64
            dstap = bass.AP(
                feats.tensor,
                feats.offset + base,
                [list(feats.ap[0]), [(q1 - q0) * 64, 2], [1, 64]],
            )
            nc.scalar.activation(dstap, pf[:], act_copy)

    # ---------------- Stage D: conv ----------------
    for T in range(NT):
        po = p2pool.tile([128, F], F32, name="pout")
        n = 0
        for r in range(3):
            for j in range(3):
                base = j * 12288 + (T * 6 + r * 2) * 64
                lhsT = feats[:, base : base + 128]
                nc.tensor.matmul(
                    po[:],
                    lhsT,
                    kmat[:, (r * 3 + j) * F : (r * 3 + j + 1) * F],
                    start=(n == 0),
                    stop=(n == 8),
                )
                n += 1
        ot = opool.tile([128, F], F32, name="ot")
        nc.vector.tensor_copy(ot[:], po[:])
        nc.sync.dma_start(out=out[T * 128 : (T + 1) * 128, :], in_=ot[:])


_CACHED_NC = None


def _get_nc():
    global _CACHED_NC
    if _CACHED_NC is None:
        _CACHED_NC = build_nc()
    return _CACHED_NC


def kernel(x, offset, modulation, conv_kernel, conv_bias):
    from concourse.bass_utils import run_bass_kernel_spmd

    B = x.shape[0]
    consts = _host_consts()
    ck9 = np.ascontiguousarray(
        conv_kernel.reshape(NK, C, F), dtype=np.float32
    )
    in_maps = []
    for b in range(B):
        in_maps.append(
            {
                "x": np.ascontiguousarray(x[b].reshape(NPIX, C), np.float32),
                "off": np.ascontiguousarray(
                    offset[b].reshape(NPIX, 2 * NK), np.float32
                ),
                "mod": np.ascontiguousarray(
                    modulation[b].reshape(NPIX, NK), np.float32
                ),
                "ck": ck9,
                "byp": consts["byp"],
                "bxp": consts["bxp"],
                "by16": consts["by16"],
                "bx16": consts["bx16"],
                "id4": np.asarray(consts["id4"]),
            }
        )
    nc = _get_nc()
    res = run_bass_kernel_spmd(
        nc,
        in_maps,
        core_ids=list(range(B)),
        trace=bool(int(os.environ.get("KERNEL_TRACE", "0"))),
    )
    outs = [res.results[b]["out"].reshape(H, W, F) for b in range(B)]
    result = np.stack(outs, axis=0) + conv_bias[None, None, None, :]
    if getattr(res, "exec_time_ns", None):
        kernel.last_exec_time_ns = res.exec_time_ns
    return result.astype(np.float32)
